# revision 29
# baseline (speedup 1.0000x reference)
"""Trainium2 Bass kernel for nn_CrossSeqTransformer (S=4, B=1, C=128, h=w=d=10).

Strategy (8 NeuronCores, sequence-parallel over L = S*N = 4000 tokens):
  - Everything runs in "transposed token space": tokensT [C=128 partitions, L free].
  - Each core owns a 500-token block (padded to 512) of the L axis: it computes
    Q / attention rows / LN / FFN only for its block, while K/V are computed
    replicated from the full token set (cheap: C=128 projections).
  - log_R bias: log(clip(r_i*r_j, eps)) == log r_i + log r_j (clip never binds
    since r in [0.1,1)); the row term cancels in softmax, so the bias reduces to
    a per-column beta*log r_j, which in the transposed layout is a per-partition
    ACT bias fused into the exp() activation. No LxL bias materialization, no
    softmax max-subtraction needed (logits are bounded ~ +-3).
  - Scores are computed transposed (S^T[j,i] tiles, j on partitions) so the PV
    matmul needs no transposes; denominators come free via a ones-column
    appended to V. One AllGather (bf16, 128x500 per core) between the two
    layers rebuilds the full token set for layer-2 K/V.
  - Matmuls run in bf16 with fp32 PSUM accumulation; the residual stream, LN
    statistics and softmax accumulation stay fp32.

Hardware-constraint notes: every engine AP must start at a 32-aligned
partition, so heads (Dh=16) are stored padded to 32 partitions/columns, and PV
outputs land at PSUM partition {0,32,64,96} via explicit tile_position.
"""

import numpy as np
import ml_dtypes

S, B, C, N = 4, 1, 128, 1000
L = S * N                    # 4000
H, DH = 8, 16
NL = 2
NCORE = 8
BLK = L // NCORE             # 500
BLKP = 512                   # padded block
LP = 4096                    # padded L
NJT = LP // 128              # 32 j-tiles
SCALE = DH ** -0.5           # 0.25
COL = 500                    # live columns of the 512-wide block
LN_EPS = 1e-5
PAD_BIAS = -30.0             # exp(-30) ~ 1e-13: padding tokens vanish

_CACHE = {}

# Schraudolph bf16-exp constants: bf16_bits(exp(x)) ~= int16(x*128*log2e + B)
LOG2E = 1.4426950408889634
SCH_A = 128.0 * LOG2E            # 184.664
SCH_B = 128.0 * 127.0 - 5.5      # bias optimised for round-to-nearest

# exp-engine split between ACT (table exp) and DVE (Schraudolph int16
# trick), weighted by measured per-group costs (ACT 1113ns vs DVE 1284ns).
def _exp_assignment(n, w_dve=13, period=28):
    out = []
    err = 0
    for _ in range(n):
        err += w_dve
        if err >= period:
            err -= period
            out.append("D")
        else:
            out.append("A")
    return out


def _pin_act_tables():
    """Make Exp/Ln resolvable only via natural_log_exp_and_others so the
    act-table-load pass doesn't thrash between exp/ln/sqrt sets (each
    switch costs ~2.7us). Set ids stay positional, only membership of the
    pruned sets changes."""
    import concourse.bacc as bacc
    import concourse.mybir as mybir
    if getattr(bacc, "_act_tables_pinned", False):
        return
    orig = bacc.get_activation_tables

    def patched(arch):
        t = orig(arch)
        exp_f = mybir.ActivationFunctionType.Exp
        ln_f = mybir.ActivationFunctionType.Ln
        out = {}
        for name, funcs in t.items():
            if name != "natural_log_exp_and_others":
                funcs = funcs - {exp_f, ln_f}
            out[name] = funcs
        return out

    bacc.get_activation_tables = patched
    bacc._act_tables_pinned = True


def _enable_ldw_opt():
    import concourse.bass_utils as bu
    if getattr(bu, "_ldw_opt_patched", False):
        return
    orig = bu.bir_verify_and_optimise

    def patched(*a, **k):
        import unittest.mock as _m
        return orig(*a, **k)

    # swap the hardcoded flag by wrapping run_command
    orig_rc = bu.run_command

    def rc(cmd, **kw):
        cmd = ["--enable-ldw-opt=true" if c == "--enable-ldw-opt=false" else c
               for c in cmd]
        return orig_rc(cmd, **kw)

    bu.run_command = rc
    bu._ldw_opt_patched = True


def _build_nc():
    import concourse.bacc as bacc
    import concourse.mybir as mybir
    import concourse.tile as tile

    _pin_act_tables()

    F32 = mybir.dt.float32
    BF16 = mybir.dt.bfloat16
    I16 = mybir.dt.int16
    AF = mybir.ActivationFunctionType
    OP = mybir.AluOpType

    nc = bacc.Bacc("TRN2", target_bir_lowering=False, debug=False,
                   num_devices=NCORE)

    # ---- DRAM I/O ----------------------------------------------------------
    d_z = nc.dram_tensor("z3", [S, C, N], BF16, kind="ExternalInput").ap()
    d_zblk = nc.dram_tensor("zblk", [C, BLKP], F32, kind="ExternalInput").ap()
    d_seqe = nc.dram_tensor("seqe", [C, 1], F32, kind="ExternalInput").ap()
    d_seqeT = nc.dram_tensor("seqeT", [C, S], F32, kind="ExternalInput").ap()
    d_wk = nc.dram_tensor("wkp", [NL, 2, C, C], BF16, kind="ExternalInput").ap()
    d_wq = nc.dram_tensor("wqp", [NL, 2, C, C], BF16, kind="ExternalInput").ap()
    d_wv = nc.dram_tensor("wvp", [NL, C, 256], BF16, kind="ExternalInput").ap()
    d_wp = nc.dram_tensor("wpp", [NL, 2, C, C], BF16, kind="ExternalInput").ap()
    d_logrb = nc.dram_tensor("logrb", [C, NL * NJT], F32, kind="ExternalInput").ap()
    d_schb = nc.dram_tensor("schb", [C, NL * NJT], F32, kind="ExternalInput").ap()
    d_w1 = nc.dram_tensor("w1", [C, 4 * C], BF16, kind="ExternalInput").ap()
    d_w2 = nc.dram_tensor("w2", [4 * C, C], BF16, kind="ExternalInput").ap()
    d_b1t = nc.dram_tensor("b1t", [C, 4], F32, kind="ExternalInput").ap()
    d_b2 = nc.dram_tensor("b2c", [C, 1], F32, kind="ExternalInput").ap()
    d_ln1gn = nc.dram_tensor("ln1gn", [C, NL], F32, kind="ExternalInput").ap()
    d_ln1b = nc.dram_tensor("ln1b", [C, NL], F32, kind="ExternalInput").ap()
    d_ln2gn = nc.dram_tensor("ln2gn", [C, 1], F32, kind="ExternalInput").ap()
    d_ln2b = nc.dram_tensor("ln2b", [C, 1], F32, kind="ExternalInput").ap()
    d_out = nc.dram_tensor("outT", [C, BLK], F32, kind="ExternalOutput").ap()

    # expansion matrix: row m contributes iff m == 32*(m'//32)+16 (denominator row)
    E = np.zeros((128, 128), np.float32)
    for mp in range(128):
        E[32 * (mp // 32) + 16, mp] = 1.0
    d_emat = nc.inline_tensor(E.astype(ml_dtypes.bfloat16), name="emat")
    d_ones = nc.inline_tensor(np.full((128, 128), 1.0 / 128, np.float32),
                              name="ones128")

    from concourse import bass_isa
    RED = bass_isa.ReduceOp

    with tile.TileContext(nc, num_cores=NCORE, pool_alloc_mode="queue") as tc:
        with tc.tile_pool(name="sb", bufs=1) as sb, \
             tc.tile_pool(name="sx", bufs=3) as sx, \
             tc.tile_pool(name="se", bufs=10) as se, \
             tc.tile_pool(name="pss", bufs=3, space="PSUM") as pss, \
             tc.tile_pool(name="psa", bufs=2, space="PSUM") as psa, \
             tc.tile_pool(name="dram", bufs=1, space="DRAM") as dram:

            # ---- resident tiles -------------------------------------------
            tokT = sb.tile([128, LP], BF16)       # layer-1 tokens^T (padded)
            tokT2 = sb.tile([128, LP], BF16)      # layer-2 tokens^T
            ktA = sb.tile([128, LP], BF16)        # K^T heads 0-3 (32-padded)
            ktB = sb.tile([128, LP], BF16)        # K^T heads 4-7
            qtA = sb.tile([128, BLKP], BF16)
            qtB = sb.tile([128, BLKP], BF16)
            vp = sb.tile([128, NJT * 256], BF16)  # V' per j-tile, head-padded
            xblk = sb.tile([128, BLKP], F32)      # residual stream (block)
            xblkb = sb.tile([128, BLKP], BF16)    # bf16 copy for matmuls

            wks = sb.tile([128, NL * 2 * 128], BF16)
            wqs = sb.tile([128, NL * 2 * 128], BF16)
            wvs = sb.tile([128, NL * 256], BF16)
            wps = sb.tile([128, NL * 2 * 128], BF16)
            w1s = sb.tile([128, 512], BF16)
            w2s = sb.tile([128, 512], BF16)
            emat = sb.tile([128, 128], BF16)
            onesc = sb.tile([128, 128], F32)
            zeroc = sb.tile([128, 1], F32)
            logrb = sb.tile([128, NL * NJT], F32)
            schb = sb.tile([128, NL * NJT], F32)
            b1t = sb.tile([128, 4], F32)
            b2c = sb.tile([128, 1], F32)
            ln1gn = sb.tile([128, NL], F32)
            ln1b = sb.tile([128, NL], F32)
            ln2gn = sb.tile([128, 1], F32)
            ln2b = sb.tile([128, 1], F32)
            seqeT = sb.tile([128, S], F32)
            seqe = sb.tile([128, 1], F32)
            epsc = sb.tile([128, 1], F32)
            g4 = sb.tile([128, 2048], BF16)       # gelu(hdn^T) chunks

            # ---- load weights ---------------------------------------------
            zstage = sx.tile([128, BLKP], F32, tag="zstage")
            nc.gpsimd.dma_start(zstage[:], d_zblk)
            nc.gpsimd.dma_start(w1s[:], d_w1)
            nc.gpsimd.dma_start(
                w2s[:].rearrange("p (f c) -> p f c", f=4),
                d_w2.rearrange("(f p) c -> p f c", p=128))
            nc.gpsimd.dma_start(emat[:], d_emat.ap())
            nc.gpsimd.dma_start(onesc[:], d_ones.ap())
            nc.vector.memset(zeroc[:], 0.0)
            nc.gpsimd.dma_start(logrb[:], d_logrb)
            nc.gpsimd.dma_start(schb[:], d_schb)
            nc.gpsimd.dma_start(b1t[:], d_b1t)
            nc.gpsimd.dma_start(b2c[:], d_b2)
            nc.scalar.dma_start(ln1gn[:], d_ln1gn)
            nc.scalar.dma_start(ln1b[:], d_ln1b)
            nc.scalar.dma_start(ln2gn[:], d_ln2gn)
            nc.scalar.dma_start(ln2b[:], d_ln2b)
            nc.scalar.dma_start(seqeT[:], d_seqeT)
            nc.scalar.dma_start(seqe[:], d_seqe)
            nc.vector.memset(epsc[:], LN_EPS)

            def wk_l(l, t):
                return wks[:, (l * 2 + t) * 128:(l * 2 + t + 1) * 128]

            def wq_l(l, t):
                return wqs[:, (l * 2 + t) * 128:(l * 2 + t + 1) * 128]

            def wp_l(l, t):
                return wps[:, (l * 2 + t) * 128:(l * 2 + t + 1) * 128]

            # NOTE: no PE warmup / duty-filler matmuls.  The HAM is a power
            # credit system: idle time refills k=8/8 duty credit, sustained
            # work drains it to k=4/8.  Dummy matmuls burn credit that the
            # real sweep needs; the only lever is less PE work.

            # ---- tokenize: tokT = z^T + seq_embed^T; xblk = zblk + seqe ----
            # Tokenize is the critical path into the K/V projections, so it
            # runs split across Vector and Scalar (GpSimd takes ~14us per
            # 1000-wide tensor_scalar — 15x slower).  Emitted BEFORE the vp
            # memsets so the Vector queue reaches it immediately.
            stages = []
            for s in range(S):
                stage = sx.tile([128, N], BF16, tag="stage")
                (nc.sync if s % 2 == 0 else nc.scalar).dma_start(stage[:], d_z[s])
                stages.append(stage)
            # weight DMAs on the hardware-DGE queues AFTER the z stages so
            # tokenize unblocks first; Q/K projections need them ~2us later
            nc.sync.dma_start(
                wks[:].rearrange("p (l t b) -> p l t b", l=NL, t=2),
                d_wk.rearrange("l t a b -> a l t b"))
            nc.scalar.dma_start(
                wqs[:].rearrange("p (l t b) -> p l t b", l=NL, t=2),
                d_wq.rearrange("l t a b -> a l t b"))
            nc.sync.dma_start(
                wvs[:].rearrange("p (l b) -> p l b", l=NL),
                d_wv.rearrange("l a b -> a l b"))
            nc.scalar.dma_start(
                wps[:].rearrange("p (l t b) -> p l t b", l=NL, t=2),
                d_wp.rearrange("l t a b -> a l t b"))
            for s in range(S):
                nc.vector.tensor_scalar_add(tokT[:, s * N:(s + 1) * N],
                                            stages[s][:], seqeT[:, s:s + 1])
            nc.gpsimd.memset(tokT[:, L:LP], 0.0)
            nc.vector.tensor_scalar_add(xblk[:], zstage[:], seqe[:])
            nc.scalar.copy(xblkb[:], xblk[:])
            # vp zero/ones pattern is layer-invariant: build once, after the
            # tokenize ops in the Vector queue (only needed by V-copies).
            vp4 = vp[:].rearrange("p (j h c) -> p j h c", j=NJT, h=H)
            nc.vector.memset(vp[:], 0.0)
            nc.vector.memset(vp4[:, :, :, 16:17], 1.0)
            # zero the 12 pad i-columns of Q so the ihalf=1 score waves can
            # run a uniform 256-wide stream (pad scores land in output cols
            # 500-511, which nothing reads)
            nc.vector.memset(qtA[:, COL:BLKP], 0.0)
            nc.vector.memset(qtB[:, COL:BLKP], 0.0)

            def layernorm_cols(y, out_f32, out_bf16, gneg, bvec, c0, c1):
                """out = g*(y-mu)*rstd + b over partitions (C) for columns
                [c0:c1); colsums via PE ones-matmul, rstd = exp(-0.5*ln(var+
                eps)) on ACT."""
                w = c1 - c0
                psL = psa.tile([128, BLKP], F32, tag="acc", name="psL")
                psL2 = psa.tile([128, BLKP], F32, tag="acc", name="psL2")
                nc.tensor.matmul(psL[:, 0:w], onesc[:], y[:, c0:c1],
                                 start=True, stop=True)       # mu (broadcast)
                t = sx.tile([128, BLKP], F32, tag="ln1", name="t")
                nc.vector.tensor_sub(t[:, 0:w], psL[:, 0:w], y[:, c0:c1])
                sq = sx.tile([128, BLKP], F32, tag="ln2", name="sq")
                nc.vector.tensor_mul(sq[:, 0:w], t[:, 0:w], t[:, 0:w])
                nc.tensor.matmul(psL2[:, 0:w], onesc[:], sq[:, 0:w],
                                 start=True, stop=True)       # var (broadcast)
                lnv = sx.tile([128, BLKP], F32, tag="ln0", name="lnv")
                nc.scalar.activation(lnv[:, 0:w], psL2[:, 0:w],
                                     AF.Ln, bias=epsc[:], scale=1.0)
                rstd = sx.tile([128, BLKP], F32, tag="ln2", name="rstd")
                nc.scalar.activation(rstd[:, 0:w], lnv[:, 0:w],
                                     AF.Exp, bias=zeroc[:], scale=-0.5)
                ts = sx.tile([128, BLKP], F32, tag="ln0", name="ts")
                nc.vector.tensor_mul(ts[:, 0:w], t[:, 0:w], rstd[:, 0:w])
                nc.vector.tensor_scalar(out=out_f32[:, c0:c1],
                                        in0=ts[:, 0:w],
                                        scalar1=gneg, scalar2=bvec,
                                        op0=OP.mult, op1=OP.add)
                nc.vector.tensor_copy(out_bf16[:, c0:c1], out_f32[:, c0:c1])

            def layernorm(y, out_f32, out_bf16, gneg, bvec):
                for (c0, c1) in ((0, 256), (256, COL)):
                    layernorm_cols(y, out_f32, out_bf16, gneg, bvec, c0, c1)

            for l in range(NL):
                tok = tokT if l == 0 else tokT2

                # ---- Q projections (tiny, first) --------------------------
                for t, qt in ((0, qtA), (1, qtB)):
                    psQ = pss.tile([128, 1024], F32, tag="s", name="psQ")
                    nc.tensor.matmul(psQ[:, 0:COL], wq_l(l, t),
                                     xblkb[:, 0:COL], start=True, stop=True)
                    (nc.vector.tensor_copy if t == 0 else nc.scalar.copy)(
                        qt[:, 0:COL], psQ[:, 0:COL])

                psOutA = psa.tile([128, BLKP], F32, tag="acc")
                psOutB = psa.tile([128, BLKP], F32, tag="acc")
                stageN = {}

                def stage_n_chain(k, psO, splits=1):
                    a = sx.tile([128, BLKP], BF16, tag=f"aoN{k}", name=f"aoN{k}")
                    t = sx.tile([128, BLKP], BF16, tag=f"aoX{k}", name=f"aoX{k}")
                    bounds = ((0, COL),) if splits == 1 else ((0, 256), (256, COL))
                    for (c0, c1) in bounds:
                        w = c1 - c0
                        nc.vector.tensor_copy(t[:, c0:c1], psO[:, c0:c1])
                        d = pss.tile([128, 1024], F32, tag="s", name=f"psD{k}")
                        nc.tensor.matmul(d[:, 0:w], emat[:], t[:, c0:c1],
                                         start=True, stop=True)
                        ln_ = sx.tile([128, BLKP], F32, tag=f"rp{k}",
                                      name=f"lnd{k}")
                        nc.scalar.activation(ln_[:, 0:w], d[:, 0:w],
                                             AF.Ln, bias=zeroc[:], scale=1.0)
                        r = sx.tile([128, BLKP], F32, tag=f"rp{k}", name=f"rp{k}")
                        nc.scalar.activation(r[:, 0:w], ln_[:, 0:w],
                                             AF.Exp, bias=zeroc[:], scale=-1.0)
                        nc.vector.tensor_mul(a[:, c0:c1], t[:, c0:c1],
                                             r[:, 0:w])
                    stageN[k] = a

                # ---- quad500 attention super-groups -----------------------
                # sg = (quad, jt): 4 concurrent 500-wide score MMs into the
                # 4 distinct PSUM banks of TWO [128,1024] psS tiles
                # (concurrent full-partition PSUM writers must hit distinct
                # banks) -> 2 exps of [128,1024] (one head-pair each, split
                # ACT/DVE; the wide unit amortizes the ~380ns per-instruction
                # overhead) -> 4 col-banded PV MMs (disjoint partitions).
                # PVs lag one sg so the PE never waits on a fresh exp.
                exp_units = _exp_assignment(2 * NJT * 2)
                uctr = [0]

                def emit_scores(quad, jt):
                    kt, qt = (ktA, qtA) if quad == 0 else (ktB, qtB)
                    tiles = []
                    for pair in range(2):
                        psS = pss.tile([128, 1024], F32, tag="s", name="psS")
                        for i in range(2):
                            q = 2 * pair + i
                            nc.tensor.matmul(
                                psS[:, 512 * i:512 * i + COL],
                                kt[32 * q:32 * q + 16, 128 * jt:128 * (jt + 1)],
                                qt[32 * q:32 * q + 16, 0:COL],
                                start=True, stop=True,
                                tile_position=(32 * q, 0))
                        tiles.append(psS)
                    return tiles

                def emit_exps(quad, jt, tiles):
                    col = l * NJT + jt
                    rhss = []
                    for pair in range(2):
                        psS = tiles[pair]
                        eng = exp_units[uctr[0] % len(exp_units)]
                        uctr[0] += 1
                        if eng == "A":
                            expS = se.tile([128, 1024], BF16, tag="ea",
                                           name="expA")
                            nc.scalar.activation(
                                expS[:, 0:1024], psS[:, 0:1024], AF.Exp,
                                bias=logrb[:, col:col + 1], scale=SCALE)
                            rhss += [expS[:, 512 * i:512 * i + COL]
                                     for i in range(2)]
                        else:
                            expS = se.tile([128, 1024], I16, tag="ed",
                                           name="expD")
                            nc.vector.tensor_scalar(
                                out=expS[:, 0:1024], in0=psS[:, 0:1024],
                                scalar1=float(SCH_A * SCALE),
                                scalar2=schb[:, col:col + 1],
                                op0=OP.mult, op1=OP.add)
                            rhss += [expS[:, 512 * i:512 * i + COL].bitcast(BF16)
                                     for i in range(2)]
                    return rhss

                def emit_pvs(quad, jt, rhss):
                    psO = psOutA if quad == 0 else psOutB
                    for q in range(4):
                        h = 4 * quad + q
                        nc.tensor.matmul(
                            psO[32 * q:32 * q + 32, 0:COL],
                            vp[:, 256 * jt + 32 * h:256 * jt + 32 * h + 32],
                            rhss[q],
                            start=(jt == 0), stop=(jt == NJT - 1),
                            tile_position=(0, 32 * q))

                pend = []

                def push_sg(quad, jt):
                    tiles = emit_scores(quad, jt)
                    rhss = emit_exps(quad, jt, tiles)
                    pend.append((quad, jt, rhss))
                    if len(pend) > 1:
                        gq, gj, grhss = pend.pop(0)
                        emit_pvs(gq, gj, grhss)
                        if (gq, gj) == (0, NJT - 1):
                            stage_n_chain(0, psOutA)

                def flush_sgs():
                    while pend:
                        gq, gj, grhss = pend.pop(0)
                        emit_pvs(gq, gj, grhss)
                        if (gq, gj) == (0, NJT - 1):
                            stage_n_chain(0, psOutA)

                # ---- quad-0 sweep with K/V projections interleaved --------
                # Projections are spread BETWEEN sg pushes in sub-us chunks:
                # a >2us PE detour (K chunk + 8 V tiles at a block boundary)
                # outruns the ~1-sg exp lookahead and starves ACT/DVE for
                # ~4.7us per c4 block (measured).
                def emit_K_chunk(t, kt, c4):
                    psK = pss.tile([128, 1024], F32, tag="s", name="psK")
                    for hw_ in range(2):
                        nc.tensor.matmul(
                            psK[:, 512 * hw_:512 * (hw_ + 1)], wk_l(l, t),
                            tok[:, 1024 * c4 + 512 * hw_:
                                 1024 * c4 + 512 * (hw_ + 1)],
                            start=True, stop=True)
                    (nc.scalar.copy if (c4 + t) % 2 == 0 else
                     nc.vector.tensor_copy)(
                        kt[:, 1024 * c4:1024 * (c4 + 1)], psK[:, 0:1024])

                def emit_V(jt):
                    psV = pss.tile([128, 1024], F32, tag="s", name="psV")
                    nc.tensor.matmul(psV[:, 0:256],
                                     tok[:, 128 * jt:128 * (jt + 1)],
                                     wvs[:, l * 256:(l + 1) * 256],
                                     start=True, stop=True)
                    psV4 = psV[:, 0:256].rearrange("p (h c) -> p h c", h=H)
                    eng_copy = nc.scalar.copy if jt % 2 == 0 else \
                        nc.vector.tensor_copy
                    eng_copy(vp4[:, jt, :, 0:16], psV4[:, :, 0:16])

                for c4 in range(4):
                    jts = list(range(8 * c4, 8 * c4 + 8))
                    emit_K_chunk(0, ktA, c4)
                    for idx, jt in enumerate(jts):
                        emit_V(jt)
                        if idx == 3:
                            emit_K_chunk(1, ktB, c4)
                        if idx >= 1:
                            push_sg(0, jts[idx - 1])
                    push_sg(0, jts[7])
                # ---- quad-1 sweep (all inputs ready) ----------------------
                for jt in range(NJT):
                    push_sg(1, jt)
                flush_sgs()

                # ---- normalize by denominators + output projection --------
                stage_n_chain(1, psOutB, splits=2)
                psP = psa.tile([128, BLKP], F32, tag="acc")
                nc.tensor.matmul(psP[:, 0:COL], wp_l(l, 0),
                                 stageN[0][:, 0:COL], start=True, stop=False)
                nc.tensor.matmul(psP[:, 0:COL], wp_l(l, 1),
                                 stageN[1][:, 0:COL], start=False, stop=True)

                # ---- residual + LN1 ---------------------------------------
                y = sx.tile([128, BLKP], F32, tag="y")
                for (c0, c1) in ((0, 256), (256, COL)):
                    nc.vector.tensor_add(y[:, c0:c1], psP[:, c0:c1],
                                         xblk[:, c0:c1])
                layernorm(y, xblk, xblkb,
                          ln1gn[:, l:l + 1], ln1b[:, l:l + 1])

                # ---- AllGather of updated tokens between layers -----------
                # Split into two half-block collectives: 64KB/rank keeps the
                # runtime in the low-latency mesh regime (vs RDH at 128KB),
                # and AG#1 fires while LN1's second column chunk computes.
                if l == 0:
                    HB = BLK // 2
                    ag_in = dram.tile([2, 128, HB], BF16, tag="agin")
                    ag_outs = [
                        dram.tile([NCORE * 128, HB], BF16, addr_space="Shared",
                                  tag=f"agout{h2}", name=f"ag_out{h2}")
                        for h2 in range(2)]
                    for h2 in range(2):
                        nc.sync.dma_start(ag_in[h2], xblkb[:, h2 * HB:(h2 + 1) * HB])
                        nc.gpsimd.collective_compute(
                            "AllGather", OP.bypass,
                            replica_groups=[list(range(NCORE))],
                            ins=[ag_in[h2].opt()], outs=[ag_outs[h2].opt()])
                        ag_v = ag_outs[h2].rearrange("(r c) n -> r c n", r=NCORE)
                        for rr in range(NCORE):
                            eng = nc.sync if rr % 2 == 0 else nc.scalar
                            eng.dma_start(
                                tokT2[:, BLK * rr + h2 * HB:
                                      BLK * rr + (h2 + 1) * HB], ag_v[rr])
                    nc.gpsimd.memset(tokT2[:, L:LP], 0.0)

            # ---- FFN + LN2 ------------------------------------------------
            # h-projection per LN1 column chunk: chunk (0,256) work starts
            # while LN1 still computes (256,500)
            for (c0, c1) in ((0, 256), (256, COL)):
                for f in range(4):
                    psH = pss.tile([128, 1024], F32, tag="s")
                    nc.tensor.matmul(psH[:, 0:c1 - c0],
                                     w1s[:, 128 * f:128 * (f + 1)],
                                     xblkb[:, c0:c1], start=True, stop=True)
                    nc.scalar.activation(g4[:, 512 * f + c0:512 * f + c1],
                                         psH[:, 0:c1 - c0], AF.Gelu,
                                         bias=b1t[:, f:f + 1], scale=1.0)
            psF = psa.tile([128, BLKP], F32, tag="acc")
            for (c0, c1) in ((0, 256), (256, COL)):
                for f in range(4):
                    nc.tensor.matmul(psF[:, c0:c1],
                                     w2s[:, 128 * f:128 * (f + 1)],
                                     g4[:, 512 * f + c0:512 * f + c1],
                                     start=(f == 0), stop=(f == 3))
            y2 = sx.tile([128, BLKP], F32, tag="y")
            for (c0, c1) in ((0, 256), (256, COL)):
                nc.vector.scalar_tensor_tensor(out=y2[:, c0:c1],
                                               in0=psF[:, c0:c1],
                                               scalar=b2c[:],
                                               in1=xblk[:, c0:c1],
                                               op0=OP.add, op1=OP.add)
            final = sx.tile([128, BLKP], F32, tag="fin")
            finb = sx.tile([128, BLKP], BF16, tag="finb")
            layernorm(y2, final, finb, ln2gn[:], ln2b[:])
            nc.sync.dma_start(d_out[:, 0:256], final[:, 0:256])
            nc.scalar.dma_start(d_out[:, 256:BLK], final[:, 256:BLK])

    nc.compile()
    return nc


def _prep_inputs(z, r, seq_embed, Wq, Wk, Wv, Wp, beta, ln1_g, ln1_b,
                 ffn_w1, ffn_b1, ffn_w2, ffn_b2, ln2_g, ln2_b):
    """Host-side data layout prep (slicing, padding, small transposes)."""
    bf = ml_dtypes.bfloat16
    f32 = np.float32
    z3f = np.asarray(z, f32).reshape(S, C, N)
    z3 = np.ascontiguousarray(z3f).astype(ml_dtypes.bfloat16)
    seqeT = np.ascontiguousarray(np.asarray(seq_embed, f32).T)      # [C,S]

    def pad_heads(W):
        # [C, C] -> [C, 128] with head q at cols 32q..32q+15, rest zero
        Wp_ = np.zeros((C, 128), f32)
        for q in range(4):
            Wp_[:, 32 * q:32 * q + 16] = W[:, 16 * q:16 * q + 16]
        return Wp_

    wkp = np.zeros((NL, 2, C, C), f32)
    wqp = np.zeros((NL, 2, C, C), f32)
    wpp = np.zeros((NL, 2, C, C), f32)
    wvp = np.zeros((NL, C, 256), f32)
    for l in range(NL):
        for t in range(2):
            wkp[l, t] = pad_heads(np.asarray(Wk[l], f32)[:, 64 * t:64 * t + 64])
            wqp[l, t] = pad_heads(np.asarray(Wq[l], f32)[:, 64 * t:64 * t + 64])
            # WpPad: rows 32q+d = Wp[16(4t+q)+d, :]
            for q in range(4):
                wpp[l, t, 32 * q:32 * q + 16, :] = np.asarray(Wp[l], f32)[
                    16 * (4 * t + q):16 * (4 * t + q) + 16, :]
        for h in range(H):
            wvp[l, :, 32 * h:32 * h + 16] = np.asarray(Wv[l], f32)[:, 16 * h:16 * h + 16]

    logr = np.log(np.asarray(r, np.float64)).astype(f32)            # [S]
    logr_tok = np.repeat(logr, N)                                   # [L]
    logrb = np.full((128, NL * NJT), PAD_BIAS, f32)
    for l in range(NL):
        bl = float(np.asarray(beta, f32)[l])
        col = np.full(LP, PAD_BIAS, f32)
        col[:L] = bl * logr_tok
        logrb[:, l * NJT:(l + 1) * NJT] = col.reshape(NJT, 128).T
    # Schraudolph bias table: z = s*(SCH_A*scale) + (SCH_A*beta*logr + SCH_B)
    schb = (SCH_A * logrb + np.float32(SCH_B)).astype(f32)
    b1t = np.ascontiguousarray(np.asarray(ffn_b1, f32).reshape(4, C).T)
    b2c = np.asarray(ffn_b2, f32).reshape(C, 1)
    ln1gn = np.ascontiguousarray(-np.asarray(ln1_g, f32).T)         # [C,NL]
    ln1bT = np.ascontiguousarray(np.asarray(ln1_b, f32).T)
    ln2gn = (-np.asarray(ln2_g, f32)).reshape(C, 1)
    ln2bc = np.asarray(ln2_b, f32).reshape(C, 1)

    common = {
        "z3": z3,
        "seqeT": seqeT,
        "wkp": wkp.astype(bf), "wqp": wqp.astype(bf),
        "wvp": wvp.astype(bf), "wpp": wpp.astype(bf),
        "logrb": logrb,
        "schb": schb,
        "w1": np.asarray(ffn_w1, f32).astype(bf),
        "w2": np.asarray(ffn_w2, f32).astype(bf),
        "b1t": b1t, "b2c": b2c,
        "ln1gn": ln1gn, "ln1b": ln1bT,
        "ln2gn": ln2gn, "ln2b": ln2bc,
    }
    in_maps = []
    for k in range(NCORE):
        s = (k * BLK) // N
        off = (k * BLK) % N
        zblk = np.zeros((C, BLKP), f32)
        zblk[:, 0:BLK] = z3f[s, :, off:off + BLK]
        m = dict(common)
        m["zblk"] = zblk
        m["seqe"] = np.ascontiguousarray(seqeT[:, s:s + 1])
        in_maps.append(m)
    return in_maps


def kernel(z, r, seq_embed, Wq, Wk, Wv, Wp, beta, ln1_g, ln1_b,
           ffn_w1, ffn_b1, ffn_w2, ffn_b2, ln2_g, ln2_b, _results_out=None):
    from concourse.bass_utils import run_bass_kernel_spmd

    if "nc" not in _CACHE:
        _CACHE["nc"] = _build_nc()
    nc = _CACHE["nc"]

    in_maps = _prep_inputs(z, r, seq_embed, Wq, Wk, Wv, Wp, beta,
                           ln1_g, ln1_b, ffn_w1, ffn_b1, ffn_w2, ffn_b2,
                           ln2_g, ln2_b)
    res = run_bass_kernel_spmd(nc, in_maps, core_ids=list(range(NCORE)))
    if _results_out is not None:
        _results_out.append(res)

    blocks = [res.results[k]["outT"] for k in range(NCORE)]   # [C, BLK] each
    big = np.stack(blocks, axis=0).reshape(S, 2, C, BLK)
    tokTfin = big.transpose(0, 2, 1, 3).reshape(S, C, N)      # [s, c, n]
    out = tokTfin.reshape(1, S * C, 10, 10, 10).astype(np.float32)
    return out



# revision 30
# speedup vs baseline: 1.0870x; 1.0870x over previous
"""Trainium2 Bass kernel for nn_CrossSeqTransformer (S=4, B=1, C=128, h=w=d=10).

Strategy (8 NeuronCores, sequence-parallel over L = S*N = 4000 tokens):
  - Everything runs in "transposed token space": tokensT [C=128 partitions, L free].
  - Each core owns a 500-token block (padded to 512) of the L axis: it computes
    Q / attention rows / LN / FFN only for its block, while K/V are computed
    replicated from the full token set (cheap: C=128 projections).
  - log_R bias: log(clip(r_i*r_j, eps)) == log r_i + log r_j (clip never binds
    since r in [0.1,1)); the row term cancels in softmax, so the bias reduces to
    a per-column beta*log r_j, which in the transposed layout is a per-partition
    ACT bias fused into the exp() activation. No LxL bias materialization, no
    softmax max-subtraction needed (logits are bounded ~ +-3).
  - Scores are computed transposed (S^T[j,i] tiles, j on partitions) so the PV
    matmul needs no transposes; denominators come free via a ones-column
    appended to V. One AllGather (bf16, 128x500 per core) between the two
    layers rebuilds the full token set for layer-2 K/V.
  - Matmuls run in bf16 with fp32 PSUM accumulation; the residual stream, LN
    statistics and softmax accumulation stay fp32.

Hardware-constraint notes: every engine AP must start at a 32-aligned
partition, so heads (Dh=16) are stored padded to 32 partitions/columns, and PV
outputs land at PSUM partition {0,32,64,96} via explicit tile_position.
"""

import numpy as np
import ml_dtypes

S, B, C, N = 4, 1, 128, 1000
L = S * N                    # 4000
H, DH = 8, 16
NL = 2
NCORE = 8
BLK = L // NCORE             # 500
BLKP = 512                   # padded block
LP = 4096                    # padded L
NJT = LP // 128              # 32 j-tiles
SCALE = DH ** -0.5           # 0.25
COL = 500                    # live columns of the 512-wide block
LN_EPS = 1e-5
PAD_BIAS = -30.0             # exp(-30) ~ 1e-13: padding tokens vanish

_CACHE = {}
INTERLEAVE = False

# Schraudolph bf16-exp constants: bf16_bits(exp(x)) ~= int16(x*128*log2e + B)
LOG2E = 1.4426950408889634
SCH_A = 128.0 * LOG2E            # 184.664
SCH_B = 128.0 * 127.0 - 5.5      # bias optimised for round-to-nearest

# exp-engine split between ACT (table exp) and DVE (Schraudolph int16
# trick), weighted by measured per-group costs (ACT 1113ns vs DVE 1284ns).
def _exp_assignment(n, w_dve=13, period=28):
    out = []
    err = 0
    for _ in range(n):
        err += w_dve
        if err >= period:
            err -= period
            out.append("D")
        else:
            out.append("A")
    return out


def _pin_act_tables():
    """Make Exp/Ln resolvable only via natural_log_exp_and_others so the
    act-table-load pass doesn't thrash between exp/ln/sqrt sets (each
    switch costs ~2.7us). Set ids stay positional, only membership of the
    pruned sets changes."""
    import concourse.bacc as bacc
    import concourse.mybir as mybir
    if getattr(bacc, "_act_tables_pinned", False):
        return
    orig = bacc.get_activation_tables

    def patched(arch):
        t = orig(arch)
        exp_f = mybir.ActivationFunctionType.Exp
        ln_f = mybir.ActivationFunctionType.Ln
        out = {}
        for name, funcs in t.items():
            if name != "natural_log_exp_and_others":
                funcs = funcs - {exp_f, ln_f}
            out[name] = funcs
        return out

    bacc.get_activation_tables = patched
    bacc._act_tables_pinned = True


def _enable_ldw_opt():
    import concourse.bass_utils as bu
    if getattr(bu, "_ldw_opt_patched", False):
        return
    orig = bu.bir_verify_and_optimise

    def patched(*a, **k):
        import unittest.mock as _m
        return orig(*a, **k)

    # swap the hardcoded flag by wrapping run_command
    orig_rc = bu.run_command

    def rc(cmd, **kw):
        cmd = ["--enable-ldw-opt=true" if c == "--enable-ldw-opt=false" else c
               for c in cmd]
        return orig_rc(cmd, **kw)

    bu.run_command = rc
    bu._ldw_opt_patched = True


def _build_nc():
    import concourse.bacc as bacc
    import concourse.mybir as mybir
    import concourse.tile as tile

    _pin_act_tables()

    F32 = mybir.dt.float32
    BF16 = mybir.dt.bfloat16
    I16 = mybir.dt.int16
    AF = mybir.ActivationFunctionType
    OP = mybir.AluOpType

    nc = bacc.Bacc("TRN2", target_bir_lowering=False, debug=False,
                   num_devices=NCORE)

    # ---- DRAM I/O ----------------------------------------------------------
    d_z = nc.dram_tensor("z3", [S, C, N], BF16, kind="ExternalInput").ap()
    d_zblk = nc.dram_tensor("zblk", [C, BLKP], F32, kind="ExternalInput").ap()
    d_seqe = nc.dram_tensor("seqe", [C, 1], F32, kind="ExternalInput").ap()
    d_seqeT = nc.dram_tensor("seqeT", [C, S], F32, kind="ExternalInput").ap()
    d_wk = nc.dram_tensor("wkp", [NL, 2, C, C], BF16, kind="ExternalInput").ap()
    d_wq = nc.dram_tensor("wqp", [NL, 2, C, C], BF16, kind="ExternalInput").ap()
    d_wv = nc.dram_tensor("wvp", [NL, C, 256], BF16, kind="ExternalInput").ap()
    d_wp = nc.dram_tensor("wpp", [NL, 2, C, C], BF16, kind="ExternalInput").ap()
    d_logrb = nc.dram_tensor("logrb", [C, NL * NJT], F32, kind="ExternalInput").ap()
    d_schb = nc.dram_tensor("schb", [C, NL * NJT], F32, kind="ExternalInput").ap()
    d_w1 = nc.dram_tensor("w1", [C, 4 * C], BF16, kind="ExternalInput").ap()
    d_w2 = nc.dram_tensor("w2", [4 * C, C], BF16, kind="ExternalInput").ap()
    d_b1t = nc.dram_tensor("b1t", [C, 4], F32, kind="ExternalInput").ap()
    d_b2 = nc.dram_tensor("b2c", [C, 1], F32, kind="ExternalInput").ap()
    d_ln1gn = nc.dram_tensor("ln1gn", [C, NL], F32, kind="ExternalInput").ap()
    d_ln1b = nc.dram_tensor("ln1b", [C, NL], F32, kind="ExternalInput").ap()
    d_ln2gn = nc.dram_tensor("ln2gn", [C, 1], F32, kind="ExternalInput").ap()
    d_ln2b = nc.dram_tensor("ln2b", [C, 1], F32, kind="ExternalInput").ap()
    d_out = nc.dram_tensor("outT", [C, BLK], F32, kind="ExternalOutput").ap()

    # expansion matrix: row m contributes iff m == 32*(m'//32)+16 (denominator row)
    E = np.zeros((128, 128), np.float32)
    for mp in range(128):
        E[32 * (mp // 32) + 16, mp] = 1.0
    d_emat = nc.inline_tensor(E.astype(ml_dtypes.bfloat16), name="emat")
    d_ones = nc.inline_tensor(np.full((128, 128), 1.0 / 128, np.float32),
                              name="ones128")

    from concourse import bass_isa
    RED = bass_isa.ReduceOp

    with tile.TileContext(nc, num_cores=NCORE, pool_alloc_mode="queue") as tc:
        with tc.tile_pool(name="sb", bufs=1) as sb, \
             tc.tile_pool(name="sx", bufs=3) as sx, \
             tc.tile_pool(name="se", bufs=10) as se, \
             tc.tile_pool(name="pss", bufs=3, space="PSUM") as pss, \
             tc.tile_pool(name="psa", bufs=2, space="PSUM") as psa, \
             tc.tile_pool(name="dram", bufs=1, space="DRAM") as dram:

            # ---- resident tiles -------------------------------------------
            tokT = sb.tile([128, LP], BF16)       # layer-1 tokens^T (padded)
            tokT2 = sb.tile([128, LP], BF16)      # layer-2 tokens^T
            ktA = sb.tile([128, LP], BF16)        # K^T heads 0-3 (32-padded)
            ktB = sb.tile([128, LP], BF16)        # K^T heads 4-7
            qtA = sb.tile([128, BLKP], BF16)
            qtB = sb.tile([128, BLKP], BF16)
            vp = sb.tile([128, NJT * 256], BF16)  # V' per j-tile, head-padded
            xblk = sb.tile([128, BLKP], F32)      # residual stream (block)
            xblkb = sb.tile([128, BLKP], BF16)    # bf16 copy for matmuls

            wks = sb.tile([128, NL * 2 * 128], BF16)
            wqs = sb.tile([128, NL * 2 * 128], BF16)
            wvs = sb.tile([128, NL * 256], BF16)
            wps = sb.tile([128, NL * 2 * 128], BF16)
            w1s = sb.tile([128, 512], BF16)
            w2s = sb.tile([128, 512], BF16)
            emat = sb.tile([128, 128], BF16)
            onesc = sb.tile([128, 128], F32)
            zeroc = sb.tile([128, 1], F32)
            logrb = sb.tile([128, NL * NJT], F32)
            schb = sb.tile([128, NL * NJT], F32)
            b1t = sb.tile([128, 4], F32)
            b2c = sb.tile([128, 1], F32)
            ln1gn = sb.tile([128, NL], F32)
            ln1b = sb.tile([128, NL], F32)
            ln2gn = sb.tile([128, 1], F32)
            ln2b = sb.tile([128, 1], F32)
            seqeT = sb.tile([128, S], F32)
            seqe = sb.tile([128, 1], F32)
            epsc = sb.tile([128, 1], F32)
            g4 = sb.tile([128, 2048], BF16)       # gelu(hdn^T) chunks

            # ---- load weights ---------------------------------------------
            zstage = sx.tile([128, BLKP], F32, tag="zstage")
            nc.gpsimd.dma_start(zstage[:], d_zblk)
            nc.gpsimd.dma_start(w1s[:], d_w1)
            nc.gpsimd.dma_start(
                w2s[:].rearrange("p (f c) -> p f c", f=4),
                d_w2.rearrange("(f p) c -> p f c", p=128))
            nc.gpsimd.dma_start(emat[:], d_emat.ap())
            nc.gpsimd.dma_start(onesc[:], d_ones.ap())
            nc.vector.memset(zeroc[:], 0.0)
            nc.gpsimd.dma_start(logrb[:], d_logrb)
            nc.gpsimd.dma_start(schb[:], d_schb)
            nc.gpsimd.dma_start(b1t[:], d_b1t)
            nc.gpsimd.dma_start(b2c[:], d_b2)
            nc.scalar.dma_start(ln1gn[:], d_ln1gn)
            nc.scalar.dma_start(ln1b[:], d_ln1b)
            nc.scalar.dma_start(ln2gn[:], d_ln2gn)
            nc.scalar.dma_start(ln2b[:], d_ln2b)
            nc.scalar.dma_start(seqeT[:], d_seqeT)
            nc.scalar.dma_start(seqe[:], d_seqe)
            nc.vector.memset(epsc[:], LN_EPS)

            def wk_l(l, t):
                return wks[:, (l * 2 + t) * 128:(l * 2 + t + 1) * 128]

            def wq_l(l, t):
                return wqs[:, (l * 2 + t) * 128:(l * 2 + t + 1) * 128]

            def wp_l(l, t):
                return wps[:, (l * 2 + t) * 128:(l * 2 + t + 1) * 128]

            # NOTE: no PE warmup / duty-filler matmuls.  The HAM is a power
            # credit system: idle time refills k=8/8 duty credit, sustained
            # work drains it to k=4/8.  Dummy matmuls burn credit that the
            # real sweep needs; the only lever is less PE work.

            # ---- tokenize: tokT = z^T + seq_embed^T; xblk = zblk + seqe ----
            # Tokenize is the critical path into the K/V projections, so it
            # runs split across Vector and Scalar (GpSimd takes ~14us per
            # 1000-wide tensor_scalar — 15x slower).  Emitted BEFORE the vp
            # memsets so the Vector queue reaches it immediately.
            stages = []
            for s in range(S):
                stage = sx.tile([128, N], BF16, tag="stage")
                (nc.sync if s % 2 == 0 else nc.scalar).dma_start(stage[:], d_z[s])
                stages.append(stage)
            # weight DMAs on the hardware-DGE queues AFTER the z stages so
            # tokenize unblocks first; Q/K projections need them ~2us later
            nc.sync.dma_start(
                wks[:].rearrange("p (l t b) -> p l t b", l=NL, t=2),
                d_wk.rearrange("l t a b -> a l t b"))
            nc.scalar.dma_start(
                wqs[:].rearrange("p (l t b) -> p l t b", l=NL, t=2),
                d_wq.rearrange("l t a b -> a l t b"))
            nc.sync.dma_start(
                wvs[:].rearrange("p (l b) -> p l b", l=NL),
                d_wv.rearrange("l a b -> a l b"))
            nc.scalar.dma_start(
                wps[:].rearrange("p (l t b) -> p l t b", l=NL, t=2),
                d_wp.rearrange("l t a b -> a l t b"))
            for s in range(S):
                nc.vector.tensor_scalar_add(tokT[:, s * N:(s + 1) * N],
                                            stages[s][:], seqeT[:, s:s + 1])
            nc.gpsimd.memset(tokT[:, L:LP], 0.0)
            nc.vector.tensor_scalar_add(xblk[:], zstage[:], seqe[:])
            nc.scalar.copy(xblkb[:], xblk[:])
            # vp zero/ones pattern is layer-invariant: build once, after the
            # tokenize ops in the Vector queue (only needed by V-copies).
            vp4 = vp[:].rearrange("p (j h c) -> p j h c", j=NJT, h=H)
            nc.vector.memset(vp[:], 0.0)
            nc.vector.memset(vp4[:, :, :, 16:17], 1.0)
            # zero the 12 pad i-columns of Q so the ihalf=1 score waves can
            # run a uniform 256-wide stream (pad scores land in output cols
            # 500-511, which nothing reads)
            nc.vector.memset(qtA[:, COL:BLKP], 0.0)
            nc.vector.memset(qtB[:, COL:BLKP], 0.0)

            def layernorm_cols(y, out_f32, out_bf16, gneg, bvec, c0, c1):
                """out = g*(y-mu)*rstd + b over partitions (C) for columns
                [c0:c1); colsums via PE ones-matmul, rstd = exp(-0.5*ln(var+
                eps)) on ACT."""
                w = c1 - c0
                psL = psa.tile([128, BLKP], F32, tag="acc", name="psL")
                psL2 = psa.tile([128, BLKP], F32, tag="acc", name="psL2")
                nc.tensor.matmul(psL[:, 0:w], onesc[:], y[:, c0:c1],
                                 start=True, stop=True)       # mu (broadcast)
                t = sx.tile([128, BLKP], F32, tag="ln1", name="t")
                nc.vector.tensor_sub(t[:, 0:w], psL[:, 0:w], y[:, c0:c1])
                sq = sx.tile([128, BLKP], F32, tag="ln2", name="sq")
                nc.vector.tensor_mul(sq[:, 0:w], t[:, 0:w], t[:, 0:w])
                nc.tensor.matmul(psL2[:, 0:w], onesc[:], sq[:, 0:w],
                                 start=True, stop=True)       # var (broadcast)
                lnv = sx.tile([128, BLKP], F32, tag="ln0", name="lnv")
                nc.scalar.activation(lnv[:, 0:w], psL2[:, 0:w],
                                     AF.Ln, bias=epsc[:], scale=1.0)
                rstd = sx.tile([128, BLKP], F32, tag="ln2", name="rstd")
                nc.scalar.activation(rstd[:, 0:w], lnv[:, 0:w],
                                     AF.Exp, bias=zeroc[:], scale=-0.5)
                ts = sx.tile([128, BLKP], F32, tag="ln0", name="ts")
                nc.vector.tensor_mul(ts[:, 0:w], t[:, 0:w], rstd[:, 0:w])
                nc.vector.tensor_scalar(out=out_f32[:, c0:c1],
                                        in0=ts[:, 0:w],
                                        scalar1=gneg, scalar2=bvec,
                                        op0=OP.mult, op1=OP.add)
                nc.vector.tensor_copy(out_bf16[:, c0:c1], out_f32[:, c0:c1])

            def layernorm(y, out_f32, out_bf16, gneg, bvec):
                for (c0, c1) in ((0, 256), (256, COL)):
                    layernorm_cols(y, out_f32, out_bf16, gneg, bvec, c0, c1)

            for l in range(NL):
                tok = tokT if l == 0 else tokT2

                # ---- Q projections (tiny, first) --------------------------
                for t, qt in ((0, qtA), (1, qtB)):
                    psQ = pss.tile([128, 1024], F32, tag="s", name="psQ")
                    nc.tensor.matmul(psQ[:, 0:COL], wq_l(l, t),
                                     xblkb[:, 0:COL], start=True, stop=True)
                    (nc.vector.tensor_copy if t == 0 else nc.scalar.copy)(
                        qt[:, 0:COL], psQ[:, 0:COL])

                psOutA = psa.tile([128, BLKP], F32, tag="acc")
                psOutB = psa.tile([128, BLKP], F32, tag="acc")
                stageN = {}

                def stage_n_chain(k, psO, splits=1):
                    a = sx.tile([128, BLKP], BF16, tag=f"aoN{k}", name=f"aoN{k}")
                    t = sx.tile([128, BLKP], BF16, tag=f"aoX{k}", name=f"aoX{k}")
                    bounds = ((0, COL),) if splits == 1 else ((0, 256), (256, COL))
                    for (c0, c1) in bounds:
                        w = c1 - c0
                        nc.vector.tensor_copy(t[:, c0:c1], psO[:, c0:c1])
                        d = pss.tile([128, 1024], F32, tag="s", name=f"psD{k}")
                        nc.tensor.matmul(d[:, 0:w], emat[:], t[:, c0:c1],
                                         start=True, stop=True)
                        ln_ = sx.tile([128, BLKP], F32, tag=f"rp{k}",
                                      name=f"lnd{k}")
                        nc.scalar.activation(ln_[:, 0:w], d[:, 0:w],
                                             AF.Ln, bias=zeroc[:], scale=1.0)
                        r = sx.tile([128, BLKP], F32, tag=f"rp{k}", name=f"rp{k}")
                        nc.scalar.activation(r[:, 0:w], ln_[:, 0:w],
                                             AF.Exp, bias=zeroc[:], scale=-1.0)
                        nc.vector.tensor_mul(a[:, c0:c1], t[:, c0:c1],
                                             r[:, 0:w])
                    stageN[k] = a

                # ---- quad500 attention super-groups -----------------------
                # sg = (quad, jt): 4 concurrent 500-wide score MMs into the
                # 4 distinct PSUM banks of TWO [128,1024] psS tiles
                # (concurrent full-partition PSUM writers must hit distinct
                # banks) -> 2 exps of [128,1024] (one head-pair each, split
                # ACT/DVE; the wide unit amortizes the ~380ns per-instruction
                # overhead) -> 4 col-banded PV MMs (disjoint partitions).
                # PVs lag one sg so the PE never waits on a fresh exp.
                exp_units = _exp_assignment(2 * NJT * 2)
                uctr = [0]

                def emit_scores(quad, jt):
                    kt, qt = (ktA, qtA) if quad == 0 else (ktB, qtB)
                    tiles = []
                    for pair in range(2):
                        psS = pss.tile([128, 1024], F32, tag="s", name="psS")
                        for i in range(2):
                            q = 2 * pair + i
                            nc.tensor.matmul(
                                psS[:, 512 * i:512 * i + COL],
                                kt[32 * q:32 * q + 16, 128 * jt:128 * (jt + 1)],
                                qt[32 * q:32 * q + 16, 0:COL],
                                start=True, stop=True,
                                tile_position=(32 * q, 0))
                        tiles.append(psS)
                    return tiles

                def emit_exps(quad, jt, tiles):
                    col = l * NJT + jt
                    rhss = []
                    for pair in range(2):
                        psS = tiles[pair]
                        eng = exp_units[uctr[0] % len(exp_units)]
                        uctr[0] += 1
                        if eng == "A":
                            expS = se.tile([128, 1024], BF16, tag="ea",
                                           name="expA")
                            nc.scalar.activation(
                                expS[:, 0:1024], psS[:, 0:1024], AF.Exp,
                                bias=logrb[:, col:col + 1], scale=SCALE)
                            rhss += [expS[:, 512 * i:512 * i + COL]
                                     for i in range(2)]
                        else:
                            expS = se.tile([128, 1024], I16, tag="ed",
                                           name="expD")
                            nc.vector.tensor_scalar(
                                out=expS[:, 0:1024], in0=psS[:, 0:1024],
                                scalar1=float(SCH_A * SCALE),
                                scalar2=schb[:, col:col + 1],
                                op0=OP.mult, op1=OP.add)
                            rhss += [expS[:, 512 * i:512 * i + COL].bitcast(BF16)
                                     for i in range(2)]
                    return rhss

                def emit_pvs(quad, jt, rhss):
                    psO = psOutA if quad == 0 else psOutB
                    for q in range(4):
                        h = 4 * quad + q
                        nc.tensor.matmul(
                            psO[32 * q:32 * q + 32, 0:COL],
                            vp[:, 256 * jt + 32 * h:256 * jt + 32 * h + 32],
                            rhss[q],
                            start=(jt == 0), stop=(jt == NJT - 1),
                            tile_position=(0, 32 * q))

                pend = []

                def push_sg(quad, jt):
                    tiles = emit_scores(quad, jt)
                    rhss = emit_exps(quad, jt, tiles)
                    pend.append((quad, jt, rhss))
                    if len(pend) > 1:
                        gq, gj, grhss = pend.pop(0)
                        emit_pvs(gq, gj, grhss)
                        if (gq, gj) == (0, NJT - 1):
                            stage_n_chain(0, psOutA)

                def flush_sgs():
                    while pend:
                        gq, gj, grhss = pend.pop(0)
                        emit_pvs(gq, gj, grhss)
                        if (gq, gj) == (0, NJT - 1):
                            stage_n_chain(0, psOutA)

                # ---- quad-0 sweep with K/V projections interleaved --------
                # Projections are spread BETWEEN sg pushes in sub-us chunks:
                # a >2us PE detour (K chunk + 8 V tiles at a block boundary)
                # outruns the ~1-sg exp lookahead and starves ACT/DVE for
                # ~4.7us per c4 block (measured).
                def emit_K_chunk(t, kt, c4):
                    psK = pss.tile([128, 1024], F32, tag="s", name="psK")
                    for hw_ in range(2):
                        nc.tensor.matmul(
                            psK[:, 512 * hw_:512 * (hw_ + 1)], wk_l(l, t),
                            tok[:, 1024 * c4 + 512 * hw_:
                                 1024 * c4 + 512 * (hw_ + 1)],
                            start=True, stop=True)
                    (nc.scalar.copy if (c4 + t) % 2 == 0 else
                     nc.vector.tensor_copy)(
                        kt[:, 1024 * c4:1024 * (c4 + 1)], psK[:, 0:1024])

                def emit_V(jt):
                    psV = pss.tile([128, 1024], F32, tag="s", name="psV")
                    nc.tensor.matmul(psV[:, 0:256],
                                     tok[:, 128 * jt:128 * (jt + 1)],
                                     wvs[:, l * 256:(l + 1) * 256],
                                     start=True, stop=True)
                    psV4 = psV[:, 0:256].rearrange("p (h c) -> p h c", h=H)
                    eng_copy = nc.scalar.copy if jt % 2 == 0 else \
                        nc.vector.tensor_copy
                    eng_copy(vp4[:, jt, :, 0:16], psV4[:, :, 0:16])

                if INTERLEAVE:
                    for c4 in range(4):
                        jts = list(range(8 * c4, 8 * c4 + 8))
                        emit_K_chunk(0, ktA, c4)
                        for idx, jt in enumerate(jts):
                            emit_V(jt)
                            if idx == 3:
                                emit_K_chunk(1, ktB, c4)
                            if idx >= 1:
                                push_sg(0, jts[idx - 1])
                        push_sg(0, jts[7])
                else:
                    for c4 in range(4):
                        emit_K_chunk(0, ktA, c4)
                        emit_K_chunk(1, ktB, c4)
                        for jt in range(8 * c4, 8 * c4 + 8):
                            emit_V(jt)
                        for jt in range(8 * c4, 8 * c4 + 8):
                            push_sg(0, jt)
                # ---- quad-1 sweep (all inputs ready) ----------------------
                for jt in range(NJT):
                    push_sg(1, jt)
                flush_sgs()

                # ---- normalize by denominators + output projection --------
                stage_n_chain(1, psOutB, splits=2)
                psP = psa.tile([128, BLKP], F32, tag="acc")
                nc.tensor.matmul(psP[:, 0:COL], wp_l(l, 0),
                                 stageN[0][:, 0:COL], start=True, stop=False)
                nc.tensor.matmul(psP[:, 0:COL], wp_l(l, 1),
                                 stageN[1][:, 0:COL], start=False, stop=True)

                # ---- residual + LN1 ---------------------------------------
                y = sx.tile([128, BLKP], F32, tag="y")
                for (c0, c1) in ((0, 256), (256, COL)):
                    nc.vector.tensor_add(y[:, c0:c1], psP[:, c0:c1],
                                         xblk[:, c0:c1])
                layernorm(y, xblk, xblkb,
                          ln1gn[:, l:l + 1], ln1b[:, l:l + 1])

                # ---- AllGather of updated tokens between layers -----------
                # Split into two half-block collectives: 64KB/rank keeps the
                # runtime in the low-latency mesh regime (vs RDH at 128KB),
                # and AG#1 fires while LN1's second column chunk computes.
                if l == 0:
                    HB = BLK // 2
                    ag_in = dram.tile([2, 128, HB], BF16, tag="agin")
                    ag_outs = [
                        dram.tile([NCORE * 128, HB], BF16, addr_space="Shared",
                                  tag=f"agout{h2}", name=f"ag_out{h2}")
                        for h2 in range(2)]
                    for h2 in range(2):
                        nc.sync.dma_start(ag_in[h2], xblkb[:, h2 * HB:(h2 + 1) * HB])
                        nc.gpsimd.collective_compute(
                            "AllGather", OP.bypass,
                            replica_groups=[list(range(NCORE))],
                            ins=[ag_in[h2].opt()], outs=[ag_outs[h2].opt()])
                        ag_v = ag_outs[h2].rearrange("(r c) n -> r c n", r=NCORE)
                        for rr in range(NCORE):
                            eng = nc.sync if rr % 2 == 0 else nc.scalar
                            eng.dma_start(
                                tokT2[:, BLK * rr + h2 * HB:
                                      BLK * rr + (h2 + 1) * HB], ag_v[rr])
                    nc.gpsimd.memset(tokT2[:, L:LP], 0.0)

            # ---- FFN + LN2 ------------------------------------------------
            # h-projection per LN1 column chunk: chunk (0,256) work starts
            # while LN1 still computes (256,500)
            for (c0, c1) in ((0, 256), (256, COL)):
                for f in range(4):
                    psH = pss.tile([128, 1024], F32, tag="s")
                    nc.tensor.matmul(psH[:, 0:c1 - c0],
                                     w1s[:, 128 * f:128 * (f + 1)],
                                     xblkb[:, c0:c1], start=True, stop=True)
                    nc.scalar.activation(g4[:, 512 * f + c0:512 * f + c1],
                                         psH[:, 0:c1 - c0], AF.Gelu,
                                         bias=b1t[:, f:f + 1], scale=1.0)
            psF = psa.tile([128, BLKP], F32, tag="acc")
            for (c0, c1) in ((0, 256), (256, COL)):
                for f in range(4):
                    nc.tensor.matmul(psF[:, c0:c1],
                                     w2s[:, 128 * f:128 * (f + 1)],
                                     g4[:, 512 * f + c0:512 * f + c1],
                                     start=(f == 0), stop=(f == 3))
            y2 = sx.tile([128, BLKP], F32, tag="y")
            for (c0, c1) in ((0, 256), (256, COL)):
                nc.vector.scalar_tensor_tensor(out=y2[:, c0:c1],
                                               in0=psF[:, c0:c1],
                                               scalar=b2c[:],
                                               in1=xblk[:, c0:c1],
                                               op0=OP.add, op1=OP.add)
            final = sx.tile([128, BLKP], F32, tag="fin")
            finb = sx.tile([128, BLKP], BF16, tag="finb")
            layernorm(y2, final, finb, ln2gn[:], ln2b[:])
            nc.sync.dma_start(d_out[:, 0:256], final[:, 0:256])
            nc.scalar.dma_start(d_out[:, 256:BLK], final[:, 256:BLK])

    nc.compile()
    return nc


def _prep_inputs(z, r, seq_embed, Wq, Wk, Wv, Wp, beta, ln1_g, ln1_b,
                 ffn_w1, ffn_b1, ffn_w2, ffn_b2, ln2_g, ln2_b):
    """Host-side data layout prep (slicing, padding, small transposes)."""
    bf = ml_dtypes.bfloat16
    f32 = np.float32
    z3f = np.asarray(z, f32).reshape(S, C, N)
    z3 = np.ascontiguousarray(z3f).astype(ml_dtypes.bfloat16)
    seqeT = np.ascontiguousarray(np.asarray(seq_embed, f32).T)      # [C,S]

    def pad_heads(W):
        # [C, C] -> [C, 128] with head q at cols 32q..32q+15, rest zero
        Wp_ = np.zeros((C, 128), f32)
        for q in range(4):
            Wp_[:, 32 * q:32 * q + 16] = W[:, 16 * q:16 * q + 16]
        return Wp_

    wkp = np.zeros((NL, 2, C, C), f32)
    wqp = np.zeros((NL, 2, C, C), f32)
    wpp = np.zeros((NL, 2, C, C), f32)
    wvp = np.zeros((NL, C, 256), f32)
    for l in range(NL):
        for t in range(2):
            wkp[l, t] = pad_heads(np.asarray(Wk[l], f32)[:, 64 * t:64 * t + 64])
            wqp[l, t] = pad_heads(np.asarray(Wq[l], f32)[:, 64 * t:64 * t + 64])
            # WpPad: rows 32q+d = Wp[16(4t+q)+d, :]
            for q in range(4):
                wpp[l, t, 32 * q:32 * q + 16, :] = np.asarray(Wp[l], f32)[
                    16 * (4 * t + q):16 * (4 * t + q) + 16, :]
        for h in range(H):
            wvp[l, :, 32 * h:32 * h + 16] = np.asarray(Wv[l], f32)[:, 16 * h:16 * h + 16]

    logr = np.log(np.asarray(r, np.float64)).astype(f32)            # [S]
    logr_tok = np.repeat(logr, N)                                   # [L]
    logrb = np.full((128, NL * NJT), PAD_BIAS, f32)
    for l in range(NL):
        bl = float(np.asarray(beta, f32)[l])
        col = np.full(LP, PAD_BIAS, f32)
        col[:L] = bl * logr_tok
        logrb[:, l * NJT:(l + 1) * NJT] = col.reshape(NJT, 128).T
    # Schraudolph bias table: z = s*(SCH_A*scale) + (SCH_A*beta*logr + SCH_B)
    schb = (SCH_A * logrb + np.float32(SCH_B)).astype(f32)
    b1t = np.ascontiguousarray(np.asarray(ffn_b1, f32).reshape(4, C).T)
    b2c = np.asarray(ffn_b2, f32).reshape(C, 1)
    ln1gn = np.ascontiguousarray(-np.asarray(ln1_g, f32).T)         # [C,NL]
    ln1bT = np.ascontiguousarray(np.asarray(ln1_b, f32).T)
    ln2gn = (-np.asarray(ln2_g, f32)).reshape(C, 1)
    ln2bc = np.asarray(ln2_b, f32).reshape(C, 1)

    common = {
        "z3": z3,
        "seqeT": seqeT,
        "wkp": wkp.astype(bf), "wqp": wqp.astype(bf),
        "wvp": wvp.astype(bf), "wpp": wpp.astype(bf),
        "logrb": logrb,
        "schb": schb,
        "w1": np.asarray(ffn_w1, f32).astype(bf),
        "w2": np.asarray(ffn_w2, f32).astype(bf),
        "b1t": b1t, "b2c": b2c,
        "ln1gn": ln1gn, "ln1b": ln1bT,
        "ln2gn": ln2gn, "ln2b": ln2bc,
    }
    in_maps = []
    for k in range(NCORE):
        s = (k * BLK) // N
        off = (k * BLK) % N
        zblk = np.zeros((C, BLKP), f32)
        zblk[:, 0:BLK] = z3f[s, :, off:off + BLK]
        m = dict(common)
        m["zblk"] = zblk
        m["seqe"] = np.ascontiguousarray(seqeT[:, s:s + 1])
        in_maps.append(m)
    return in_maps


def kernel(z, r, seq_embed, Wq, Wk, Wv, Wp, beta, ln1_g, ln1_b,
           ffn_w1, ffn_b1, ffn_w2, ffn_b2, ln2_g, ln2_b, _results_out=None):
    from concourse.bass_utils import run_bass_kernel_spmd

    if "nc" not in _CACHE:
        _CACHE["nc"] = _build_nc()
    nc = _CACHE["nc"]

    in_maps = _prep_inputs(z, r, seq_embed, Wq, Wk, Wv, Wp, beta,
                           ln1_g, ln1_b, ffn_w1, ffn_b1, ffn_w2, ffn_b2,
                           ln2_g, ln2_b)
    res = run_bass_kernel_spmd(nc, in_maps, core_ids=list(range(NCORE)))
    if _results_out is not None:
        _results_out.append(res)

    blocks = [res.results[k]["outT"] for k in range(NCORE)]   # [C, BLK] each
    big = np.stack(blocks, axis=0).reshape(S, 2, C, BLK)
    tokTfin = big.transpose(0, 2, 1, 3).reshape(S, C, N)      # [s, c, n]
    out = tokTfin.reshape(1, S * C, 10, 10, 10).astype(np.float32)
    return out

